# revision 94
# baseline (speedup 1.0000x reference)
"""Bass/Trainium2 kernel for MultiHeadAttentionWithDSA (sparse attention with
lightning-indexer top-64 key selection), sharded over 8 NeuronCores.

Sharding: core = b*4 + g  (b in {0,1} batch, g in {0..3} head-group of 4 heads).
Each core computes a partial output  ctx_g @ Wo[g*256:(g+1)*256, :]  for its
batch; the host sums the 4 partials per batch and adds the bias.

v3 design notes:
 - indexer (qi/ki projections + index scores) is TRUE fp32: the top-64
   selection must match the fp32 reference's ordering at the boundary
   (f32r flips ~2% of rows).  Attention q/k/v/o matmuls are f32r.
 - attention scores are computed s-major (scores^T[s, t]) so the exp'd probs
   feed attn@V directly; only the shared top-k mask is transposed.
 - probabilities E and V are bf16 (mask is exact 0/1; probs tolerate it);
   softmax denominator comes free via a ones-column appended to V; 1/den is
   broadcast across partitions with a rank-1 matmul.
 - emission order is engine-aware (engines execute their queues in order):
   indexer projections stream first so DVE top-k starts ~30us in; q/k/v
   projections and t-half-0 attention run during top-k; the only work gated
   on the LAST top-k chunk is the t-chunk-7 epilogue (output cols are
   disjoint, so normalization/output of t<896 never waits for chunk 7).
"""

import numpy as np

import concourse.bacc as bacc
import concourse.bass as bass
import concourse.mybir as mybir
import concourse.tile as tile
from concourse import masks
from concourse.bass_utils import run_bass_kernel_spmd

F32 = mybir.dt.float32
F32R = mybir.dt.float32r
BF16 = mybir.dt.bfloat16
MMDT = F32R
AF = mybir.ActivationFunctionType
ALU = mybir.AluOpType

B, T, D = 2, 1024, 1024
H, HD = 16, 64          # total heads, head dim
HG = 4                  # heads per core
HI, IHD = 4, 64         # index heads, index head dim
TOPK = 64
NCHUNK = T // 128       # 8 token chunks of 128
NEG = -3.0e30           # causal-invalid marker (additive, pre-topk)
SENT = -float(2.0 ** 100)  # match_replace sentinel

# score-matmul start per s-chunk: padded down so every f32r matmul window is
# >=256 cols (fp32r below 256 runs at 1/4 rate).  true (causal) start is
# sc*128; the pad region is computed and discarded.
T0 = [0, 128, 256, 256, 512, 640, 768, 768]

_NEFF_CACHE = "/var/tmp/bass-neff-cache"


def _install_neff_cache():
    """walrus compile output cache keyed on BIR hash (compiles are minutes)."""
    import hashlib
    import os
    import shutil

    import concourse.bass2jax as b2j

    if getattr(b2j, "_dsa_neff_cache_installed", False):
        return
    orig = b2j.compile_bir_kernel

    def cached(bir_json, tmpdir, neff_name="file.neff"):
        try:
            h = hashlib.sha256(
                bir_json if isinstance(bir_json, bytes) else bir_json.encode()
            ).hexdigest()[:24]
            os.makedirs(_NEFF_CACHE, exist_ok=True)
            hit = os.path.join(_NEFF_CACHE, h + ".neff")
            if os.path.exists(hit):
                dst = os.path.join(tmpdir, neff_name)
                shutil.copyfile(hit, dst)
                return dst
            neff = orig(bir_json, tmpdir, neff_name)
            shutil.copyfile(neff, hit + ".tmp")
            os.replace(hit + ".tmp", hit)
            return neff
        except OSError:
            return orig(bir_json, tmpdir, neff_name)

    b2j.compile_bir_kernel = cached
    b2j._dsa_neff_cache_installed = True


def _groups(lo, hi):
    """Split [lo, hi) into groups of width <=512 that never cross a 512-col
    PSUM bank boundary (matmul outputs must stay within one bank)."""
    out = []
    while lo < hi:
        nxt = min(hi, (lo // 512 + 1) * 512)
        out.append((lo, nxt))
        lo = nxt
    return out


def build_kernel(tc, out_ap, x_ap, wq_ap, wk_ap, wv_ap, wo_ap, wi_ap):
    """Emit the per-core kernel. DRAM tensors:
    x [1024,1024] f32, wq/wk/wv [1024,256] f32r, wo [256,1024] f32r,
    wi [1024,324] f32 = concat(Wqi[1024,256], Wki[1024,64], Ww[1024,4]).
    out [1024,1024] partial (pre-bias, pre-reduction over head groups)."""
    nc = tc.nc
    from contextlib import ExitStack
    stack = ExitStack()

    const_pool = stack.enter_context(tc.tile_pool(name="const", bufs=1))
    ident = const_pool.tile([128, 128], F32)
    masks.make_identity(nc, ident[:])
    causal = const_pool.tile([128, 128], F32)
    masks.make_causal_mask(nc, causal[:], mask_val=NEG)
    # causal01[t, s] = 1.0 where s <= t else 0.0 (t-major diag fix for mk01)
    causal01 = const_pool.tile([128, 128], F32)
    masks.make_lower_triangular(nc, causal01[:], val=1.0, diag=True)
    ones64f = const_pool.tile([1, 64], F32)
    nc.vector.memset(ones64f[:], 1.0)
    ones64 = const_pool.tile([1, 64], MMDT)
    nc.vector.tensor_copy(ones64[:], ones64f[:])
    ones32f = const_pool.tile([128, 32], F32)
    nc.vector.memset(ones32f[:], 1.0)

    w_pool = stack.enter_context(tc.tile_pool(name="weights", bufs=1))
    wq_sb = w_pool.tile([128, 8 * 256], MMDT)
    wk_sb = w_pool.tile([128, 8 * 256], MMDT)
    wv_sb = w_pool.tile([128, 8 * 256], MMDT)
    wo_sb = w_pool.tile([128, 2 * 1024], MMDT)

    act_pool = stack.enter_context(tc.tile_pool(name="acts", bufs=1))
    # v + ones column (bf16): s-chunk sc at cols sc*260, head h at h*65, col64=1
    v_sb = act_pool.tile([128, 8 * 260], BF16)
    w8 = act_pool.tile([128, 32], F32)           # softmax(x@Ww)/8, chunk i at cols 4i
    nc.vector.tensor_copy(
        v_sb[:].rearrange("p (c e) -> p c e", e=65)[:, :, 64:65],
        ones32f[:].rearrange("p (c e) -> p c e", e=1))

    tp_ps = stack.enter_context(tc.tile_pool(name="tp_ps", bufs=2, space="PSUM"))
    idx_pool = stack.enter_context(tc.tile_pool(name="idx", bufs=1))
    big_stack = ExitStack()
    mm_ps = big_stack.enter_context(tc.tile_pool(name="mm_ps", bufs=3, space="PSUM"))
    xstack = ExitStack()
    xscope = xstack.enter_context(tc.tile_pool(name="xscope", bufs=1))
    xT = xscope.tile([128, 8 * 1024], F32)    # [d-chunk j] at cols j*1024
    xTr = xscope.tile([128, 8 * 1024], MMDT)  # rounded shadow for f32r matmuls
    # indexer tensors (true fp32; freed with xscope after the idx matmuls)
    qiT = xscope.tile([128, 2 * 1024], F32)
    kiw = xscope.tile([128, 1024], F32)   # rows 0-63 kiT, 64-67 wT logits
    kiw2 = xscope.tile([128, 1024], F32)  # rows 64-127: copy of kiT (odd heads)

    # ---- Phase A: load x, transpose to xT; weight DMAs behind x ----
    with tc.tile_pool(name="xtok", bufs=2) as xtok_pool:
        for i in range(NCHUNK):
            xt = xtok_pool.tile([128, 1024], F32, tag="xtok")
            nc.sync.dma_start(xt[:], x_ap[i * 128:(i + 1) * 128, :])
            pt = mm_ps.tile([128, 1024], F32, name="pt", tag="mm")
            for bj in range(2):
                for q in range(4):
                    j = bj * 4 + q
                    nc.tensor.matmul(pt[:, j * 128:(j + 1) * 128],
                                     xt[:, j * 128:(j + 1) * 128], ident[:],
                                     is_transpose=True,
                                     start=(q == 0), stop=(q == 3))
            dst = xT[:].rearrange("p (c q) -> p c q", q=1024)[:, :, i * 128:(i + 1) * 128]
            if i % 2 == 0:
                nc.scalar.copy(dst, pt[:].rearrange("p (c q) -> p c q", q=128))
            else:
                nc.vector.tensor_copy(dst, pt[:].rearrange("p (c q) -> p c q", q=128))
    wistack = ExitStack()
    wiscope = wistack.enter_context(tc.tile_pool(name="wiscope", bufs=1))
    wi_sb = wiscope.tile([128, 8 * 324], F32)
    for j in range(8):
        nc.sync.dma_start(wi_sb[:, j * 324:(j + 1) * 324],
                          wi_ap[j * 128:(j + 1) * 128, :])
    if True:
        # f32r shadow of xT for the attention-side projections
        for j in range(8):
            src = xT[:, j * 1024:(j + 1) * 1024]
            dsr = xTr[:, j * 1024:(j + 1) * 1024]
            if j < 4:
                nc.gpsimd.tensor_copy(dsr, src)
            elif j < 6:
                nc.vector.tensor_copy(dsr, src)
            else:
                nc.scalar.copy(dsr, src)
        for (ap_, dst_) in ((wq_ap, wq_sb), (wk_ap, wk_sb), (wv_ap, wv_sb)):
            for j in range(8):
                nc.sync.dma_start(dst_[:, j * 256:(j + 1) * 256],
                                  ap_[j * 128:(j + 1) * 128, :])
        for ck in range(2):
            nc.sync.dma_start(wo_sb[:, ck * 1024:(ck + 1) * 1024],
                              wo_ap[ck * 128:(ck + 1) * 128, :])

    # ---- Phase B1: indexer projections (fp32), t-group 0 first ----
    def w_soft(i):
        """per-chunk softmax(x@Ww)/8 -> w8[:, 4i:4i+4]"""
        pw = tp_ps.tile([128, 128], F32, tag="tp")
        nc.tensor.transpose(pw[:, 0:4], kiw[64:68, i * 128:(i + 1) * 128],
                            ident[64:68, 64:68])
        wexp = idx_pool.tile([128, 4], F32, tag="wexp", bufs=2)
        wden = idx_pool.tile([128, 1], F32, tag="wden", bufs=2)
        nc.scalar.activation(wexp[:], pw[:, 0:4], AF.Exp, accum_out=wden[:])
        wrec = idx_pool.tile([128, 1], F32, tag="wrec", bufs=2)
        nc.vector.reciprocal(wrec[:], wden[:])
        nc.vector.tensor_scalar(w8[:, i * 4:(i + 1) * 4], wexp[:], wrec[:], 0.125,
                                op0=ALU.mult, op1=ALU.mult)

    # ---- Phase C1: indexer scores (fp32) + top-64 per chunk ----
    def emit_idx(i):
        n_s = (i + 1) * 128
        work = idx_pool.tile([128, n_s], F32, name=f"work{i}", tag=f"work{i}", bufs=1)
        for h in range(HI):
            m, r = h // 2, (h % 2) * 64
            dst = work if h == 0 else idx_pool.tile([128, 1024], F32, name="aw",
                                                    tag="aw", bufs=2)
            ps = mm_ps.tile([128, 1024], F32, tag="mm")
            for (g0, g1) in _groups(0, n_s):
                ki_rhs = kiw[0:64, g0:g1] if r == 0 else kiw2[64:128, g0:g1]
                nc.tensor.matmul(
                    ps[:, g0:g1],
                    qiT[r:r + 64, m * 1024 + i * 128: m * 1024 + (i + 1) * 128],
                    ki_rhs, start=True, stop=True)
            nc.scalar.activation(dst[:, 0:n_s], ps[:, 0:n_s], AF.Relu,
                                 scale=w8[:, i * 4 + h: i * 4 + h + 1])
            if h == 1:
                nc.gpsimd.tensor_tensor(work[:, i * 128:(i + 1) * 128],
                                        work[:, i * 128:(i + 1) * 128], causal[:],
                                        op=ALU.add)
            if h > 0:
                nc.gpsimd.tensor_tensor(work[:, 0:n_s], work[:, 0:n_s],
                                        dst[:, 0:n_s], op=ALU.add)
        tmax = idx_pool.tile([128, 8], F32, name="tmax", tag="tmax", bufs=2)
        for _ in range(8):
            nc.vector.max(tmax[:], work[:, 0:n_s])
            nc.vector.match_replace(work[:, 0:n_s], tmax[:], work[:, 0:n_s], SENT)
        return work

    # ---- Phase B1 + C1 interleaved by t-half: the top-k stream on DVE
    # starts as soon as the first half of qi/ki exists ----
    works = []
    for tg in range(2):
        c0, c1 = tg * 512, (tg + 1) * 512
        ps = mm_ps.tile([128, 1024], F32, name="bh", tag="mm")
        pq = mm_ps.tile([128, 1024], F32, name="pq", tag="mm")
        for j in range(8):   # interleaved ki | qi_m0 | qi_m1 contraction
            xs = xT[:, j * 1024 + c0: j * 1024 + c1]
            nc.tensor.matmul(ps[0:68, 0:512],
                             wi_sb[:, j * 324 + 256: j * 324 + 324], xs,
                             start=(j == 0), stop=(j == 7))
            nc.tensor.matmul(ps[:, 512:1024],
                             wi_sb[:, j * 324: j * 324 + 128], xs,
                             start=(j == 0), stop=(j == 7))
            nc.tensor.matmul(pq[:, 0:512],
                             wi_sb[:, j * 324 + 128: j * 324 + 256], xs,
                             start=(j == 0), stop=(j == 7))
        nc.scalar.copy(kiw[0:68, c0:c1], ps[0:68, 0:512])
        nc.sync.dma_start(kiw2[64:128, c0:c1], kiw[0:64, c0:c1])
        nc.scalar.copy(qiT[:, c0:c1], ps[:, 512:1024])
        nc.scalar.copy(qiT[:, 1024 + c0: 1024 + c1], pq[:, 0:512])
        for i in range(tg * 4, tg * 4 + 4):
            w_soft(i)
        for i in range(tg * 4, tg * 4 + 4):
            works.append(emit_idx(i))
    wistack.close()

    # ---- Phase B2: attention projections (f32r), run during top-k ----
    qT = act_pool.tile([128, 2 * 1024], MMDT)    # heads (2m,2m+1) rows, tokens free
    kT = act_pool.tile([128, 2 * 1024], MMDT)
    for m in range(2):
        for (wsb, dst) in ((wq_sb, qT), (wk_sb, kT)):
            ps = mm_ps.tile([128, 1024], F32, tag="mm")
            for tg in range(2):
                for j in range(8):
                    nc.tensor.matmul(
                        ps[:, tg * 512:(tg + 1) * 512],
                        wsb[:, j * 256 + m * 128: j * 256 + (m + 1) * 128],
                        xTr[:, j * 1024 + tg * 512: j * 1024 + (tg + 1) * 512],
                        start=(j == 0), stop=(j == 7))
            nc.scalar.copy(dst[:, m * 1024:(m + 1) * 1024], ps[:])
    for half in range(2):
        ps = mm_ps.tile([128, 1024], F32, tag="mm")
        for q in range(4):
            sc = half * 4 + q
            for j in range(8):
                nc.tensor.matmul(
                    ps[:, q * 256:q * 256 + 256],
                    xTr[:, j * 1024 + sc * 128: j * 1024 + (sc + 1) * 128],
                    wv_sb[:, j * 256:(j + 1) * 256],
                    start=(j == 0), stop=(j == 7))
        for q in range(4):
            sc = half * 4 + q
            dst = v_sb[:, sc * 260:(sc + 1) * 260]
            dst = dst.rearrange("p (h e) -> p h e", e=65)[:, :, 0:64]
            nc.scalar.copy(dst, ps[:, q * 256:(q + 1) * 256].rearrange(
                "p (h e) -> p h e", e=64))
    xstack.close()      # free xT/xTr/qiT/kiw
    big_stack.close()   # free the 2-bank mm tiles for the attention psum pools

    ctxT = act_pool.tile([128, 2 * 1024], MMDT)  # [ck] at cols ck*1024
    # transposed 0/1 top-k masks, s-major: maskT[sc][s, t - T0[sc]] (bf16)
    maskT = [act_pool.tile([128, 1024 - T0[sc]], BF16, name=f"maskT{sc}",
                           tag=f"maskT{sc}") for sc in range(NCHUNK)]

    # ---- Phase C2: 0/1 masks (t-major).  The causal01 diag fix is only
    # needed for chunk 0 (rows t>=128 always have >64 valid candidates).
    # Chunks 4-7 are emitted AFTER the t-half-0 attention so Pool's in-order
    # queue never parks tg0 work behind top-k-gated instructions. ----
    mks = {}

    def emit_mk01(i):
        n_s = (i + 1) * 128
        mk = idx_pool.tile([128, 1024], F32, name="mk", tag="mk", bufs=3)
        nc.gpsimd.tensor_scalar(mk[:, 0:n_s], works[i][:, 0:n_s], SENT, None,
                                op0=ALU.is_equal)
        if i == 0:
            nc.gpsimd.tensor_tensor(mk[:, 0:128], mk[:, 0:128], causal01[:],
                                    op=ALU.mult)
        mks[i] = mk

    for i in range(4):
        emit_mk01(i)

    # ---- mask transposes: chunk i's mk -> 128-col stripes of maskT[sc<=i] ----
    def emit_transposes(i):
        for bi in range((i + 4) // 4):
            cnt = min(i + 1, bi * 4 + 4) - bi * 4
            pt = tp_ps.tile([128, 512], F32, name="pt", tag="tp")
            for q in range(cnt):
                sc = bi * 4 + q
                nc.tensor.matmul(pt[:, q * 128:(q + 1) * 128],
                                 mks[i][:, sc * 128:(sc + 1) * 128], ident[:],
                                 is_transpose=True,
                                 start=(q == 0), stop=(q == cnt - 1))
            for q in range(cnt):
                sc = bi * 4 + q
                col = (i * 128) - T0[sc]
                nc.scalar.copy(maskT[sc][:, col:col + 128],
                               pt[:, q * 128:(q + 1) * 128])

    attn_ps = stack.enter_context(tc.tile_pool(name="attn_ps", bufs=2, space="PSUM"))
    ctx_ps = stack.enter_context(tc.tile_pool(name="ctx_ps", bufs=2, space="PSUM"))
    pcb_ps = stack.enter_context(tc.tile_pool(name="pcb_ps", bufs=1, space="PSUM"))
    attn2_pool = stack.enter_context(tc.tile_pool(name="attn2", bufs=2))

    def scores_exp(h, sc, lo_true, hi):
        """score matmul (f32r, padded to >=256) + exp -> bf16 E tile.
        E tile cols are [lo_true, hi)."""
        m, r = h // 2, (h % 2) * 64
        # pad the matmul window down to >=256 cols (f32r is 1/4 rate below)
        lo_mm = max(hi - 512, min(lo_true, hi - 256))
        et = attn2_pool.tile([128, 512], BF16, name=f"E{sc}", tag=f"E{sc}", bufs=4)
        ps = attn_ps.tile([128, 512], F32, tag="mm5")
        nc.tensor.matmul(
            ps[:, 0:hi - lo_mm],
            kT[r:r + 64, m * 1024 + sc * 128: m * 1024 + (sc + 1) * 128],
            qT[r:r + 64, m * 1024 + lo_mm: m * 1024 + hi],
            start=True, stop=True)
        nc.scalar.activation(et[:, 0:hi - lo_true], ps[:, lo_true - lo_mm:hi - lo_mm],
                             AF.Exp, scale=0.125)
        return et

    def norm_ctx(h, tg, pc, w, eng="pool"):
        """1/den broadcast + normalize pc[0:64, 0:w] into ctxT cols
        [tg*512, tg*512+w).  eng="pool": ACT copy + Pool TT (spares DVE while
        top-k runs); eng="dve": direct DVE TT from PSUM (post-top-k tail)."""
        ck, rr = h // 2, (h % 2) * 64
        rec = attn2_pool.tile([1, 512], MMDT, name="rec", tag="rec", bufs=2)
        with nc.allow_low_precision(reason="1/den at f32r precision"):
            nc.vector.reciprocal(rec[:, 0:w], pc[64:65, 0:w])
        rbc = tp_ps.tile([128, 512], F32, tag="tp")
        nc.tensor.matmul(rbc[0:64, 0:w], ones64[:], rec[:, 0:w], start=True, stop=True)
        rbs = attn2_pool.tile([64, 512], F32, name="rbs", tag="rbs", bufs=2)
        nc.scalar.copy(rbs[:, 0:w], rbc[0:64, 0:w])
        dst = ctxT[rr:rr + 64, ck * 1024 + tg * 512: ck * 1024 + tg * 512 + w]
        if eng == "pool":
            pcs = attn2_pool.tile([64, 512], F32, name="pcs", tag="pcs", bufs=2)
            nc.scalar.copy(pcs[:, 0:w], pc[0:64, 0:w])
            nc.gpsimd.tensor_tensor(dst, pcs[:, 0:w], rbs[:, 0:w], op=ALU.mult)
        else:
            nc.vector.tensor_tensor(dst, pc[0:64, 0:w], rbs[:, 0:w], op=ALU.mult)

    def outproj(i):
        for og in range(2):
            out_sb = attn2_pool.tile([128, 512], F32, name="out_sb", tag="out", bufs=2)
            ps = attn_ps.tile([128, 512], F32, tag="mm5")
            for ck in range(2):
                nc.tensor.matmul(
                    ps[:],
                    ctxT[:, ck * 1024 + i * 128: ck * 1024 + (i + 1) * 128],
                    wo_sb[:, ck * 1024 + og * 512: ck * 1024 + (og + 1) * 512],
                    start=(ck == 0), stop=(ck == 1))
            nc.scalar.copy(out_sb[:], ps[:])
            nc.sync.dma_start(out_ap[i * 128:(i + 1) * 128, og * 512:(og + 1) * 512],
                              out_sb[:])

    # ---- t-half 0 attention (gated on top-k chunks 0-3 only) ----
    for i in range(4):
        emit_transposes(i)
    pcs_tg0 = {}
    for h in range(HG):
        e_tiles = {}
        for sc in range(4):
            lo_true = sc * 128
            et = scores_exp(h, sc, lo_true, 512)
            w = 512 - lo_true
            moff = lo_true - T0[sc]
            nc.gpsimd.tensor_tensor(et[:, 0:w], et[:, 0:w],
                                    maskT[sc][:, moff:moff + w], op=ALU.mult)
            e_tiles[sc] = et
        pc = ctx_ps.tile([65, 512], F32, name="pc", tag="ctx")
        for sc in range(4):
            pcoff = sc * 128
            nc.tensor.matmul(
                pc[:, pcoff:512],
                v_sb[:, sc * 260 + h * 65: sc * 260 + (h + 1) * 65],
                e_tiles[sc][:, 0:512 - pcoff],
                start=(sc == 0), stop=(sc == 3))
        norm_ctx(h, 0, pc, 512)
    for i in range(4):
        outproj(i)
    for i in range(4, NCHUNK):
        emit_mk01(i)

    # ---- t-half 1 scores+exp (no mask dependency) ----
    e1 = {}
    for h in range(HG):
        for sc in range(NCHUNK):
            lo_true = max(sc * 128, 512)
            e1[(h, sc)] = scores_exp(h, sc, lo_true, 1024)

    for i in range(4, NCHUNK):
        emit_transposes(i)

    # ---- t-half 1, cols [512, 896): gated on top-k chunks 4-6 ----
    for h in range(HG):
        for sc in range(7):
            lo_true = max(sc * 128, 512)
            wA = 896 - lo_true
            moff = lo_true - T0[sc]
            eng = nc.gpsimd if sc < 3 else nc.vector
            eng.tensor_tensor(e1[(h, sc)][:, 0:wA], e1[(h, sc)][:, 0:wA],
                              maskT[sc][:, moff:moff + wA], op=ALU.mult)
        pc = ctx_ps.tile([65, 512], F32, name="pc", tag="ctx")
        for sc in range(7):
            lo_true = max(sc * 128, 512)
            pcoff = lo_true - 512
            nc.tensor.matmul(
                pc[:, pcoff:384],
                v_sb[:, sc * 260 + h * 65: sc * 260 + (h + 1) * 65],
                e1[(h, sc)][:, 0:384 - pcoff],
                start=(sc == 0), stop=(sc == 6))
        norm_ctx(h, 1, pc, 384, eng="dve")
    for i in range(4, 7):
        outproj(i)

    # ---- t-chunk 7 epilogue: the only work gated on the last top-k ----
    pcb = pcb_ps.tile([65, 512], F32, name="pcb", tag="pcb")
    for h in range(HG):
        for sc in range(NCHUNK):
            lo_true = max(sc * 128, 512)
            bo = 896 - lo_true          # B-part offset within the E tile
            moff = 896 - T0[sc]
            nc.vector.tensor_tensor(e1[(h, sc)][:, bo:bo + 128],
                                    e1[(h, sc)][:, bo:bo + 128],
                                    maskT[sc][:, moff:moff + 128], op=ALU.mult)
        for sc in range(NCHUNK):
            bo = 896 - max(sc * 128, 512)
            nc.tensor.matmul(
                pcb[:, h * 128:(h + 1) * 128],
                v_sb[:, sc * 260 + h * 65: sc * 260 + (h + 1) * 65],
                e1[(h, sc)][:, bo:bo + 128],
                start=(sc == 0), stop=(sc == 7))
    recB = attn2_pool.tile([1, 512], MMDT, name="recB", tag="recB", bufs=1)
    with nc.allow_low_precision(reason="1/den at f32r precision"):
        for h in range(HG):
            nc.vector.reciprocal(recB[:, h * 128:(h + 1) * 128],
                                 pcb[64:65, h * 128:(h + 1) * 128])
    rbcB = tp_ps.tile([128, 512], F32, tag="tp")
    nc.tensor.matmul(rbcB[0:64, :], ones64[:], recB[:], start=True, stop=True)
    rbsB = attn2_pool.tile([64, 512], F32, name="rbsB", tag="rbsB", bufs=1)
    nc.scalar.copy(rbsB[:], rbcB[0:64, :])
    for h in range(HG):
        ck, rr = h // 2, (h % 2) * 64
        nc.vector.tensor_tensor(
            ctxT[rr:rr + 64, ck * 1024 + 896: ck * 1024 + 1024],
            pcb[0:64, h * 128:(h + 1) * 128], rbsB[:, h * 128:(h + 1) * 128],
            op=ALU.mult)
    outproj(7)

    stack.close()


def _build_nc(loop=0):
    nc = bacc.Bacc("TRN2")
    x = nc.dram_tensor("x", [T, D], F32, kind="ExternalInput")
    wq = nc.dram_tensor("wq", [D, 256], F32R, kind="ExternalInput")
    wk = nc.dram_tensor("wk", [D, 256], F32R, kind="ExternalInput")
    wv = nc.dram_tensor("wv", [D, 256], F32R, kind="ExternalInput")
    wo = nc.dram_tensor("wo", [256, D], F32R, kind="ExternalInput")
    wi = nc.dram_tensor("wi", [D, 324], F32, kind="ExternalInput")
    out = nc.dram_tensor("out", [T, D], F32, kind="ExternalOutput")
    with tile.TileContext(nc) as tc:
        if loop:
            with tc.For_i(0, loop, 1):
                build_kernel(tc, out.ap(), x.ap(), wq.ap(), wk.ap(), wv.ap(), wo.ap(), wi.ap())
        else:
            build_kernel(tc, out.ap(), x.ap(), wq.ap(), wk.ap(), wv.ap(), wo.ap(), wi.ap())
    nc.compile()
    return nc


def kernel(x, Wq, Wk, Wv, Wo, bo, Wqi, Wki, Ww, _trace=False):
    _install_neff_cache()
    x, Wq, Wk, Wv, Wo, bo, Wqi, Wki, Ww = (
        np.ascontiguousarray(np.asarray(a, np.float32))
        for a in (x, Wq, Wk, Wv, Wo, bo, Wqi, Wki, Ww))
    nc = _build_nc()
    in_maps = _make_in_maps(x, Wq, Wk, Wv, Wo, Wqi, Wki, Ww)
    res = run_bass_kernel_spmd(nc, in_maps, core_ids=list(range(8)), trace=_trace)
    outs = [r["out"] for r in res.results]
    full = np.stack([sum(outs[b * 4:(b + 1) * 4]) + bo for b in range(B)], axis=0)
    full = full.astype(np.float32)
    if _trace:
        return full, res
    return full


def _make_in_maps(x, Wq, Wk, Wv, Wo, Wqi, Wki, Ww):
    wi = np.ascontiguousarray(np.concatenate([Wqi, Wki, Ww], axis=1))
    in_maps = []
    for b in range(B):
        for g in range(4):
            c = slice(g * 256, (g + 1) * 256)
            in_maps.append({
                "x": np.ascontiguousarray(x[b]),
                "wq": np.ascontiguousarray(Wq[:, c]),
                "wk": np.ascontiguousarray(Wk[:, c]),
                "wv": np.ascontiguousarray(Wv[:, c]),
                "wo": np.ascontiguousarray(Wo[c, :]),
                "wi": wi,
            })
    return in_maps


def bench_exec_ns(inputs, iters=10, loop=256):
    """Per-iteration device time: the kernel body loops `loop` times inside one
    NEFF; dispatch-overhead floor is subtracted via the slope between two loop
    counts. Returns ns per kernel iteration."""
    lo = max(1, loop // 8)
    t_hi = _bench_exec_wall(inputs, iters, loop)
    t_lo = _bench_exec_wall(inputs, iters, lo)
    return (t_hi - t_lo) / (loop - lo) * 1e9


def _bench_exec_wall(inputs, iters, loop):
    import time

    import jax
    from jax.experimental.shard_map import shard_map
    from jax.sharding import Mesh, NamedSharding, PartitionSpec

    import concourse.bass2jax as b2j

    _install_neff_cache()
    b2j.install_neuronx_cc_hook()
    nc = _build_nc(loop=loop)
    ins = {k: np.ascontiguousarray(np.asarray(v, np.float32)) for k, v in inputs.items()}
    in_maps = _make_in_maps(ins["x"], ins["Wq"], ins["Wk"], ins["Wv"], ins["Wo"],
                            ins["Wqi"], ins["Wki"], ins["Ww"])

    partition_name = nc.partition_id_tensor.name if nc.partition_id_tensor else None
    in_names, out_names, out_avals, zero_outs = [], [], [], []
    for alloc in nc.m.functions[0].allocations:
        if not isinstance(alloc, mybir.MemoryLocationSet):
            continue
        name = alloc.memorylocations[0].name
        if alloc.kind == "ExternalInput":
            if name != partition_name:
                in_names.append(name)
        elif alloc.kind == "ExternalOutput":
            shape = tuple(alloc.tensor_shape)
            dtype = mybir.dt.np(alloc.dtype)
            out_names.append(name)
            out_avals.append(jax.core.ShapedArray(shape, dtype))
            zero_outs.append(np.zeros(shape, dtype))
    n_params = len(in_names)
    all_in_names = list(in_names) + list(out_names)
    if partition_name is not None:
        all_in_names.append(partition_name)

    def _body(*args):
        operands = list(args)
        if partition_name is not None:
            operands.append(b2j.partition_id_tensor())
        outs = b2j._bass_exec_p.bind(
            *operands,
            out_avals=tuple(out_avals),
            in_names=tuple(all_in_names),
            out_names=tuple(out_names),
            lowering_input_output_aliases=(),
            sim_require_finite=True,
            sim_require_nnan=True,
            nc=nc,
        )
        return tuple(outs)

    n_cores = len(in_maps)
    devices = jax.devices()[:n_cores]
    mesh = Mesh(np.asarray(devices), ("core",))
    in_specs = (PartitionSpec("core"),) * (n_params + len(out_names))
    out_specs = (PartitionSpec("core"),) * len(out_names)
    fn = jax.jit(shard_map(_body, mesh=mesh, in_specs=in_specs,
                           out_specs=out_specs, check_rep=False))
    sharding = NamedSharding(mesh, PartitionSpec("core"))
    dev_args = [
        jax.device_put(
            np.concatenate([np.asarray(in_maps[c][nm]) for c in range(n_cores)], axis=0),
            sharding)
        for nm in in_names
    ] + [
        jax.device_put(np.concatenate([z] * n_cores, axis=0), sharding)
        for z in zero_outs
    ]
    r = fn(*dev_args)
    jax.block_until_ready(r)
    times = []
    for _ in range(iters):
        t0 = time.perf_counter()
        r = fn(*dev_args)
        jax.block_until_ready(r)
        times.append(time.perf_counter() - t0)
    return min(times)


if __name__ == "__main__":
    rng = np.random.default_rng(0)
    ins = {
        "x": rng.standard_normal((B, T, D)).astype(np.float32),
        "Wq": (rng.standard_normal((D, D)) * 0.02).astype(np.float32),
        "Wk": (rng.standard_normal((D, D)) * 0.02).astype(np.float32),
        "Wv": (rng.standard_normal((D, D)) * 0.02).astype(np.float32),
        "Wo": (rng.standard_normal((D, D)) * 0.02).astype(np.float32),
        "bo": np.zeros(D, np.float32),
        "Wqi": (rng.standard_normal((D, HI * IHD)) * 0.02).astype(np.float32),
        "Wki": (rng.standard_normal((D, IHD)) * 0.02).astype(np.float32),
        "Ww": (rng.standard_normal((D, HI)) * 0.02).astype(np.float32),
    }
    out = kernel(**ins)
    print("out", out.shape, out.dtype, float(np.abs(out).max()))


# revision 95
# speedup vs baseline: 1.2135x; 1.2135x over previous
"""Bass/Trainium2 kernel for MultiHeadAttentionWithDSA (sparse attention with
lightning-indexer top-64 key selection), sharded over 8 NeuronCores.

Sharding: core = b*4 + g  (b in {0,1} batch, g in {0..3} head-group of 4 heads).
Each core computes a partial output  ctx_g @ Wo[g*256:(g+1)*256, :]  for its
batch; the host sums the 4 partials per batch and adds the bias.

v3 design notes:
 - indexer (qi/ki projections + index scores) is TRUE fp32: the top-64
   selection must match the fp32 reference's ordering at the boundary
   (f32r flips ~2% of rows).  Attention q/k/v/o matmuls are f32r.
 - attention scores are computed s-major (scores^T[s, t]) so the exp'd probs
   feed attn@V directly; only the shared top-k mask is transposed.
 - probabilities E and V are bf16 (mask is exact 0/1; probs tolerate it);
   softmax denominator comes free via a ones-column appended to V; 1/den is
   broadcast across partitions with a rank-1 matmul.
 - emission order is engine-aware (engines execute their queues in order):
   indexer projections stream first so DVE top-k starts ~30us in; q/k/v
   projections and t-half-0 attention run during top-k; the only work gated
   on the LAST top-k chunk is the t-chunk-7 epilogue (output cols are
   disjoint, so normalization/output of t<896 never waits for chunk 7).
"""

import numpy as np

import concourse.bacc as bacc
import concourse.bass as bass
import concourse.mybir as mybir
import concourse.tile as tile
from concourse import masks
from concourse.bass_utils import run_bass_kernel_spmd

F32 = mybir.dt.float32
F32R = mybir.dt.float32r
BF16 = mybir.dt.bfloat16
MMDT = F32R
AF = mybir.ActivationFunctionType
ALU = mybir.AluOpType

B, T, D = 2, 1024, 1024
H, HD = 16, 64          # total heads, head dim
HG = 4                  # heads per core
HI, IHD = 4, 64         # index heads, index head dim
TOPK = 64
NCHUNK = T // 128       # 8 token chunks of 128
NEG = -3.0e30           # causal-invalid marker (additive, pre-topk)
SENT = -float(2.0 ** 100)  # match_replace sentinel

# score-matmul start per s-chunk: padded down so every f32r matmul window is
# >=256 cols (fp32r below 256 runs at 1/4 rate).  true (causal) start is
# sc*128; the pad region is computed and discarded.
T0 = [0, 128, 256, 256, 512, 640, 768, 768]

_NEFF_CACHE = "/var/tmp/bass-neff-cache"


def _install_neff_cache():
    """walrus compile output cache keyed on BIR hash (compiles are minutes)."""
    import hashlib
    import os
    import shutil

    import concourse.bass2jax as b2j

    if getattr(b2j, "_dsa_neff_cache_installed", False):
        return
    orig = b2j.compile_bir_kernel

    def cached(bir_json, tmpdir, neff_name="file.neff"):
        try:
            h = hashlib.sha256(
                bir_json if isinstance(bir_json, bytes) else bir_json.encode()
            ).hexdigest()[:24]
            os.makedirs(_NEFF_CACHE, exist_ok=True)
            hit = os.path.join(_NEFF_CACHE, h + ".neff")
            if os.path.exists(hit):
                dst = os.path.join(tmpdir, neff_name)
                shutil.copyfile(hit, dst)
                return dst
            neff = orig(bir_json, tmpdir, neff_name)
            shutil.copyfile(neff, hit + ".tmp")
            os.replace(hit + ".tmp", hit)
            return neff
        except OSError:
            return orig(bir_json, tmpdir, neff_name)

    b2j.compile_bir_kernel = cached
    b2j._dsa_neff_cache_installed = True


def _groups(lo, hi):
    """Split [lo, hi) into groups of width <=512 that never cross a 512-col
    PSUM bank boundary (matmul outputs must stay within one bank)."""
    out = []
    while lo < hi:
        nxt = min(hi, (lo // 512 + 1) * 512)
        out.append((lo, nxt))
        lo = nxt
    return out


def build_kernel(tc, out_ap, x_ap, wq_ap, wk_ap, wv_ap, wo_ap, wi_ap):
    """Emit the per-core kernel. DRAM tensors:
    x [1024,1024] f32, wq/wk/wv [1024,256] f32r, wo [256,1024] f32r,
    wi [1024,324] f32 = concat(Wqi[1024,256], Wki[1024,64], Ww[1024,4]).
    out [1024,1024] partial (pre-bias, pre-reduction over head groups)."""
    nc = tc.nc
    from contextlib import ExitStack
    stack = ExitStack()

    const_pool = stack.enter_context(tc.tile_pool(name="const", bufs=1))
    ident = const_pool.tile([128, 128], F32)
    masks.make_identity(nc, ident[:])
    causal = const_pool.tile([128, 128], F32)
    masks.make_causal_mask(nc, causal[:], mask_val=NEG)
    # causal01[t, s] = 1.0 where s <= t else 0.0 (t-major diag fix for mk01)
    causal01 = const_pool.tile([128, 128], F32)
    masks.make_lower_triangular(nc, causal01[:], val=1.0, diag=True)
    ones64f = const_pool.tile([1, 64], F32)
    nc.vector.memset(ones64f[:], 1.0)
    ones64 = const_pool.tile([1, 64], MMDT)
    nc.vector.tensor_copy(ones64[:], ones64f[:])
    ones32f = const_pool.tile([128, 32], F32)
    nc.vector.memset(ones32f[:], 1.0)

    w_pool = stack.enter_context(tc.tile_pool(name="weights", bufs=1))
    wq_sb = w_pool.tile([128, 8 * 256], MMDT)
    wk_sb = w_pool.tile([128, 8 * 256], MMDT)
    wv_sb = w_pool.tile([128, 8 * 256], MMDT)
    wo_sb = w_pool.tile([128, 2 * 1024], MMDT)

    act_pool = stack.enter_context(tc.tile_pool(name="acts", bufs=1))
    # v + ones column (bf16): s-chunk sc at cols sc*260, head h at h*65, col64=1
    v_sb = act_pool.tile([128, 8 * 260], BF16)
    w8 = act_pool.tile([128, 32], F32)           # softmax(x@Ww)/8, chunk i at cols 4i
    nc.vector.tensor_copy(
        v_sb[:].rearrange("p (c e) -> p c e", e=65)[:, :, 64:65],
        ones32f[:].rearrange("p (c e) -> p c e", e=1))

    tp_ps = stack.enter_context(tc.tile_pool(name="tp_ps", bufs=2, space="PSUM"))
    idx_pool = stack.enter_context(tc.tile_pool(name="idx", bufs=1))
    big_stack = ExitStack()
    mm_ps = big_stack.enter_context(tc.tile_pool(name="mm_ps", bufs=3, space="PSUM"))
    xstack = ExitStack()
    xscope = xstack.enter_context(tc.tile_pool(name="xscope", bufs=1))
    xT = xscope.tile([128, 8 * 1024], F32)    # [d-chunk j] at cols j*1024
    xTr = xscope.tile([128, 8 * 1024], MMDT)  # rounded shadow for f32r matmuls
    # indexer tensors (true fp32; freed with xscope after the idx matmuls)
    qiT = xscope.tile([128, 2 * 1024], F32)
    kiw = xscope.tile([128, 1024], F32)   # rows 0-63 kiT, 64-67 wT logits
    kiw2 = xscope.tile([128, 1024], F32)  # rows 64-127: copy of kiT (odd heads)

    # ---- Phase A: load x, transpose to xT; weight DMAs behind x ----
    with tc.tile_pool(name="xtok", bufs=2) as xtok_pool:
        for pair in range(4):
            xt = xtok_pool.tile([128, 2048], F32, tag="xtok")
            src = x_ap[pair * 256:(pair + 1) * 256, :]
            nc.sync.dma_start(xt[:].rearrange("p (c d) -> p c d", d=1024),
                              src.rearrange("(c p) d -> p c d", p=128))
            for ci in range(2):
                i = pair * 2 + ci
                pt = mm_ps.tile([128, 1024], F32, name="pt", tag="mm")
                for bj in range(2):
                    for q in range(4):
                        j = bj * 4 + q
                        nc.tensor.matmul(
                            pt[:, j * 128:(j + 1) * 128],
                            xt[:, ci * 1024 + j * 128: ci * 1024 + (j + 1) * 128],
                            ident[:], is_transpose=True,
                            start=(q == 0), stop=(q == 3))
                dst = xT[:].rearrange("p (c q) -> p c q", q=1024)[:, :, i * 128:(i + 1) * 128]
                if i % 2 == 0:
                    nc.scalar.copy(dst, pt[:].rearrange("p (c q) -> p c q", q=128))
                else:
                    nc.vector.tensor_copy(dst, pt[:].rearrange("p (c q) -> p c q", q=128))
    wistack = ExitStack()
    wiscope = wistack.enter_context(tc.tile_pool(name="wiscope", bufs=1))
    wi_sb = wiscope.tile([128, 8 * 324], F32)
    for j in range(8):
        nc.sync.dma_start(wi_sb[:, j * 324:(j + 1) * 324],
                          wi_ap[j * 128:(j + 1) * 128, :])
    if True:
        # f32r shadow of xT for the attention-side projections
        for j in range(8):
            src = xT[:, j * 1024:(j + 1) * 1024]
            dsr = xTr[:, j * 1024:(j + 1) * 1024]
            if j < 4:
                nc.gpsimd.tensor_copy(dsr, src)
            elif j < 6:
                nc.vector.tensor_copy(dsr, src)
            else:
                nc.scalar.copy(dsr, src)
        for (ap_, dst_) in ((wq_ap, wq_sb), (wk_ap, wk_sb), (wv_ap, wv_sb)):
            for j in range(8):
                nc.sync.dma_start(dst_[:, j * 256:(j + 1) * 256],
                                  ap_[j * 128:(j + 1) * 128, :])
        for ck in range(2):
            nc.sync.dma_start(wo_sb[:, ck * 1024:(ck + 1) * 1024],
                              wo_ap[ck * 128:(ck + 1) * 128, :])

    # ---- Phase B1: indexer projections (fp32), t-group 0 first ----
    def w_soft(i):
        """per-chunk softmax(x@Ww)/8 -> w8[:, 4i:4i+4]"""
        pw = tp_ps.tile([128, 128], F32, tag="tp")
        nc.tensor.transpose(pw[:, 0:4], kiw[64:68, i * 128:(i + 1) * 128],
                            ident[64:68, 64:68])
        wexp = idx_pool.tile([128, 4], F32, tag="wexp", bufs=2)
        wden = idx_pool.tile([128, 1], F32, tag="wden", bufs=2)
        nc.scalar.activation(wexp[:], pw[:, 0:4], AF.Exp, accum_out=wden[:])
        wrec = idx_pool.tile([128, 1], F32, tag="wrec", bufs=2)
        nc.vector.reciprocal(wrec[:], wden[:])
        nc.vector.tensor_scalar(w8[:, i * 4:(i + 1) * 4], wexp[:], wrec[:], 0.125,
                                op0=ALU.mult, op1=ALU.mult)

    # ---- Phase C1: indexer scores (fp32) + top-64 per chunk ----
    def emit_idx(i):
        n_s = (i + 1) * 128
        work = idx_pool.tile([128, n_s], F32, name=f"work{i}", tag=f"work{i}", bufs=1)
        for h in range(HI):
            m, r = h // 2, (h % 2) * 64
            dst = work if h == 0 else idx_pool.tile([128, 1024], F32, name="aw",
                                                    tag="aw", bufs=2)
            ps = mm_ps.tile([128, 1024], F32, tag="mm")
            for (g0, g1) in _groups(0, n_s):
                ki_rhs = kiw[0:64, g0:g1] if r == 0 else kiw2[64:128, g0:g1]
                nc.tensor.matmul(
                    ps[:, g0:g1],
                    qiT[r:r + 64, m * 1024 + i * 128: m * 1024 + (i + 1) * 128],
                    ki_rhs, start=True, stop=True)
            nc.scalar.activation(dst[:, 0:n_s], ps[:, 0:n_s], AF.Relu,
                                 scale=w8[:, i * 4 + h: i * 4 + h + 1])
            if h == 1:
                nc.gpsimd.tensor_tensor(work[:, i * 128:(i + 1) * 128],
                                        work[:, i * 128:(i + 1) * 128], causal[:],
                                        op=ALU.add)
            if h > 0:
                nc.gpsimd.tensor_tensor(work[:, 0:n_s], work[:, 0:n_s],
                                        dst[:, 0:n_s], op=ALU.add)
        tmax = idx_pool.tile([128, 8], F32, name="tmax", tag="tmax", bufs=2)
        for _ in range(8):
            nc.vector.max(tmax[:], work[:, 0:n_s])
            nc.vector.match_replace(work[:, 0:n_s], tmax[:], work[:, 0:n_s], SENT)
        return work

    # ---- Phase B1 + C1 interleaved by t-half: the top-k stream on DVE
    # starts as soon as the first half of qi/ki exists ----
    works = []
    for tg in range(2):
        c0, c1 = tg * 512, (tg + 1) * 512
        ps = mm_ps.tile([128, 1024], F32, name="bh", tag="mm")
        pq = mm_ps.tile([128, 1024], F32, name="pq", tag="mm")
        for j in range(8):   # interleaved ki | qi_m0 | qi_m1 contraction
            xs = xT[:, j * 1024 + c0: j * 1024 + c1]
            nc.tensor.matmul(ps[0:68, 0:512],
                             wi_sb[:, j * 324 + 256: j * 324 + 324], xs,
                             start=(j == 0), stop=(j == 7))
            nc.tensor.matmul(ps[:, 512:1024],
                             wi_sb[:, j * 324: j * 324 + 128], xs,
                             start=(j == 0), stop=(j == 7))
            nc.tensor.matmul(pq[:, 0:512],
                             wi_sb[:, j * 324 + 128: j * 324 + 256], xs,
                             start=(j == 0), stop=(j == 7))
        nc.scalar.copy(kiw[0:68, c0:c1], ps[0:68, 0:512])
        nc.sync.dma_start(kiw2[64:128, c0:c1], kiw[0:64, c0:c1])
        nc.scalar.copy(qiT[:, c0:c1], ps[:, 512:1024])
        nc.scalar.copy(qiT[:, 1024 + c0: 1024 + c1], pq[:, 0:512])
        for i in range(tg * 4, tg * 4 + 4):
            w_soft(i)
        for i in range(tg * 4, tg * 4 + 4):
            works.append(emit_idx(i))
    wistack.close()

    # ---- Phase B2: attention projections (f32r), run during top-k ----
    qT = act_pool.tile([128, 2 * 1024], MMDT)    # heads (2m,2m+1) rows, tokens free
    kT = act_pool.tile([128, 2 * 1024], MMDT)
    for m in range(2):
        for (wsb, dst) in ((wq_sb, qT), (wk_sb, kT)):
            ps = mm_ps.tile([128, 1024], F32, tag="mm")
            for tg in range(2):
                for j in range(8):
                    nc.tensor.matmul(
                        ps[:, tg * 512:(tg + 1) * 512],
                        wsb[:, j * 256 + m * 128: j * 256 + (m + 1) * 128],
                        xTr[:, j * 1024 + tg * 512: j * 1024 + (tg + 1) * 512],
                        start=(j == 0), stop=(j == 7))
            nc.scalar.copy(dst[:, m * 1024:(m + 1) * 1024], ps[:])
    for half in range(2):
        ps = mm_ps.tile([128, 1024], F32, tag="mm")
        for q in range(4):
            sc = half * 4 + q
            for j in range(8):
                nc.tensor.matmul(
                    ps[:, q * 256:q * 256 + 256],
                    xTr[:, j * 1024 + sc * 128: j * 1024 + (sc + 1) * 128],
                    wv_sb[:, j * 256:(j + 1) * 256],
                    start=(j == 0), stop=(j == 7))
        for q in range(4):
            sc = half * 4 + q
            dst = v_sb[:, sc * 260:(sc + 1) * 260]
            dst = dst.rearrange("p (h e) -> p h e", e=65)[:, :, 0:64]
            nc.scalar.copy(dst, ps[:, q * 256:(q + 1) * 256].rearrange(
                "p (h e) -> p h e", e=64))
    xstack.close()      # free xT/xTr/qiT/kiw
    big_stack.close()   # free the 2-bank mm tiles for the attention psum pools

    ctxT = act_pool.tile([128, 2 * 1024], MMDT)  # [ck] at cols ck*1024
    # transposed 0/1 top-k masks, s-major: maskT[sc][s, t - T0[sc]] (bf16)
    maskT = [act_pool.tile([128, 1024 - T0[sc]], BF16, name=f"maskT{sc}",
                           tag=f"maskT{sc}") for sc in range(NCHUNK)]

    # ---- Phase C2: 0/1 masks (t-major).  The causal01 diag fix is only
    # needed for chunk 0 (rows t>=128 always have >64 valid candidates).
    # Chunks 4-7 are emitted AFTER the t-half-0 attention so Pool's in-order
    # queue never parks tg0 work behind top-k-gated instructions. ----
    mks = {}

    def emit_mk01(i):
        n_s = (i + 1) * 128
        mk = idx_pool.tile([128, 1024], F32, name="mk", tag="mk", bufs=3)
        nc.gpsimd.tensor_scalar(mk[:, 0:n_s], works[i][:, 0:n_s], SENT, None,
                                op0=ALU.is_equal)
        if i == 0:
            nc.gpsimd.tensor_tensor(mk[:, 0:128], mk[:, 0:128], causal01[:],
                                    op=ALU.mult)
        mks[i] = mk

    for i in range(4):
        emit_mk01(i)

    # ---- mask transposes: chunk i's mk -> 128-col stripes of maskT[sc<=i] ----
    def emit_transposes(i):
        for bi in range((i + 4) // 4):
            cnt = min(i + 1, bi * 4 + 4) - bi * 4
            pt = tp_ps.tile([128, 512], F32, name="pt", tag="tp")
            for q in range(cnt):
                sc = bi * 4 + q
                nc.tensor.matmul(pt[:, q * 128:(q + 1) * 128],
                                 mks[i][:, sc * 128:(sc + 1) * 128], ident[:],
                                 is_transpose=True,
                                 start=(q == 0), stop=(q == cnt - 1))
            for q in range(cnt):
                sc = bi * 4 + q
                col = (i * 128) - T0[sc]
                nc.scalar.copy(maskT[sc][:, col:col + 128],
                               pt[:, q * 128:(q + 1) * 128])

    attn_ps = stack.enter_context(tc.tile_pool(name="attn_ps", bufs=2, space="PSUM"))
    ctx_ps = stack.enter_context(tc.tile_pool(name="ctx_ps", bufs=2, space="PSUM"))
    pcb_ps = stack.enter_context(tc.tile_pool(name="pcb_ps", bufs=1, space="PSUM"))
    attn2_pool = stack.enter_context(tc.tile_pool(name="attn2", bufs=2))

    def scores_exp(h, sc, lo_true, hi):
        """score matmul (f32r, padded to >=256) + exp -> bf16 E tile.
        E tile cols are [lo_true, hi)."""
        m, r = h // 2, (h % 2) * 64
        # pad the matmul window down to >=256 cols (f32r is 1/4 rate below)
        lo_mm = max(hi - 512, min(lo_true, hi - 256))
        et = attn2_pool.tile([128, 512], BF16, name=f"E{sc}", tag=f"E{sc}", bufs=4)
        ps = attn_ps.tile([128, 512], F32, tag="mm5")
        nc.tensor.matmul(
            ps[:, 0:hi - lo_mm],
            kT[r:r + 64, m * 1024 + sc * 128: m * 1024 + (sc + 1) * 128],
            qT[r:r + 64, m * 1024 + lo_mm: m * 1024 + hi],
            start=True, stop=True)
        nc.scalar.activation(et[:, 0:hi - lo_true], ps[:, lo_true - lo_mm:hi - lo_mm],
                             AF.Exp, scale=0.125)
        return et

    def norm_ctx(h, tg, pc, w, eng="pool"):
        """1/den broadcast + normalize pc[0:64, 0:w] into ctxT cols
        [tg*512, tg*512+w).  eng="pool": ACT copy + Pool TT (spares DVE while
        top-k runs); eng="dve": direct DVE TT from PSUM (post-top-k tail)."""
        ck, rr = h // 2, (h % 2) * 64
        rec = attn2_pool.tile([1, 512], MMDT, name="rec", tag="rec", bufs=2)
        with nc.allow_low_precision(reason="1/den at f32r precision"):
            nc.vector.reciprocal(rec[:, 0:w], pc[64:65, 0:w])
        rbc = tp_ps.tile([128, 512], F32, tag="tp")
        nc.tensor.matmul(rbc[0:64, 0:w], ones64[:], rec[:, 0:w], start=True, stop=True)
        rbs = attn2_pool.tile([64, 512], F32, name="rbs", tag="rbs", bufs=2)
        nc.scalar.copy(rbs[:, 0:w], rbc[0:64, 0:w])
        dst = ctxT[rr:rr + 64, ck * 1024 + tg * 512: ck * 1024 + tg * 512 + w]
        if eng == "pool":
            pcs = attn2_pool.tile([64, 512], F32, name="pcs", tag="pcs", bufs=2)
            nc.scalar.copy(pcs[:, 0:w], pc[0:64, 0:w])
            nc.gpsimd.tensor_tensor(dst, pcs[:, 0:w], rbs[:, 0:w], op=ALU.mult)
        else:
            nc.vector.tensor_tensor(dst, pc[0:64, 0:w], rbs[:, 0:w], op=ALU.mult)

    def outproj(i):
        for og in range(2):
            out_sb = attn2_pool.tile([128, 512], F32, name="out_sb", tag="out", bufs=2)
            ps = attn_ps.tile([128, 512], F32, tag="mm5")
            for ck in range(2):
                nc.tensor.matmul(
                    ps[:],
                    ctxT[:, ck * 1024 + i * 128: ck * 1024 + (i + 1) * 128],
                    wo_sb[:, ck * 1024 + og * 512: ck * 1024 + (og + 1) * 512],
                    start=(ck == 0), stop=(ck == 1))
            nc.scalar.copy(out_sb[:], ps[:])
            nc.sync.dma_start(out_ap[i * 128:(i + 1) * 128, og * 512:(og + 1) * 512],
                              out_sb[:])

    # ---- t-half 0 attention (gated on top-k chunks 0-3 only) ----
    for i in range(4):
        emit_transposes(i)
    pcs_tg0 = {}
    for h in range(HG):
        e_tiles = {}
        for sc in range(4):
            lo_true = sc * 128
            et = scores_exp(h, sc, lo_true, 512)
            w = 512 - lo_true
            moff = lo_true - T0[sc]
            nc.gpsimd.tensor_tensor(et[:, 0:w], et[:, 0:w],
                                    maskT[sc][:, moff:moff + w], op=ALU.mult)
            e_tiles[sc] = et
        pc = ctx_ps.tile([65, 512], F32, name="pc", tag="ctx")
        for sc in range(4):
            pcoff = sc * 128
            nc.tensor.matmul(
                pc[:, pcoff:512],
                v_sb[:, sc * 260 + h * 65: sc * 260 + (h + 1) * 65],
                e_tiles[sc][:, 0:512 - pcoff],
                start=(sc == 0), stop=(sc == 3))
        norm_ctx(h, 0, pc, 512)
    for i in range(4):
        outproj(i)
    for i in range(4, NCHUNK):
        emit_mk01(i)

    # ---- t-half 1 scores+exp (no mask dependency) ----
    e1 = {}
    for h in range(HG):
        for sc in range(NCHUNK):
            lo_true = max(sc * 128, 512)
            e1[(h, sc)] = scores_exp(h, sc, lo_true, 1024)

    for i in range(4, NCHUNK):
        emit_transposes(i)

    # ---- t-half 1, cols [512, 896): gated on top-k chunks 4-6 ----
    for h in range(HG):
        for sc in range(7):
            lo_true = max(sc * 128, 512)
            wA = 896 - lo_true
            moff = lo_true - T0[sc]
            eng = nc.gpsimd if sc < 3 else nc.vector
            eng.tensor_tensor(e1[(h, sc)][:, 0:wA], e1[(h, sc)][:, 0:wA],
                              maskT[sc][:, moff:moff + wA], op=ALU.mult)
        pc = ctx_ps.tile([65, 512], F32, name="pc", tag="ctx")
        for sc in range(7):
            lo_true = max(sc * 128, 512)
            pcoff = lo_true - 512
            nc.tensor.matmul(
                pc[:, pcoff:384],
                v_sb[:, sc * 260 + h * 65: sc * 260 + (h + 1) * 65],
                e1[(h, sc)][:, 0:384 - pcoff],
                start=(sc == 0), stop=(sc == 6))
        norm_ctx(h, 1, pc, 384, eng="dve")
    for i in range(4, 7):
        outproj(i)

    # ---- t-chunk 7 epilogue: the only work gated on the last top-k ----
    pcb = pcb_ps.tile([65, 512], F32, name="pcb", tag="pcb")
    for h in range(HG):
        for sc in range(NCHUNK):
            lo_true = max(sc * 128, 512)
            bo = 896 - lo_true          # B-part offset within the E tile
            moff = 896 - T0[sc]
            nc.vector.tensor_tensor(e1[(h, sc)][:, bo:bo + 128],
                                    e1[(h, sc)][:, bo:bo + 128],
                                    maskT[sc][:, moff:moff + 128], op=ALU.mult)
        for sc in range(NCHUNK):
            bo = 896 - max(sc * 128, 512)
            nc.tensor.matmul(
                pcb[:, h * 128:(h + 1) * 128],
                v_sb[:, sc * 260 + h * 65: sc * 260 + (h + 1) * 65],
                e1[(h, sc)][:, bo:bo + 128],
                start=(sc == 0), stop=(sc == 7))
    recB = attn2_pool.tile([1, 512], MMDT, name="recB", tag="recB", bufs=1)
    with nc.allow_low_precision(reason="1/den at f32r precision"):
        for h in range(HG):
            nc.vector.reciprocal(recB[:, h * 128:(h + 1) * 128],
                                 pcb[64:65, h * 128:(h + 1) * 128])
    rbcB = tp_ps.tile([128, 512], F32, tag="tp")
    nc.tensor.matmul(rbcB[0:64, :], ones64[:], recB[:], start=True, stop=True)
    rbsB = attn2_pool.tile([64, 512], F32, name="rbsB", tag="rbsB", bufs=1)
    nc.scalar.copy(rbsB[:], rbcB[0:64, :])
    for h in range(HG):
        ck, rr = h // 2, (h % 2) * 64
        nc.vector.tensor_tensor(
            ctxT[rr:rr + 64, ck * 1024 + 896: ck * 1024 + 1024],
            pcb[0:64, h * 128:(h + 1) * 128], rbsB[:, h * 128:(h + 1) * 128],
            op=ALU.mult)
    outproj(7)

    stack.close()


def _build_nc(loop=0):
    nc = bacc.Bacc("TRN2")
    x = nc.dram_tensor("x", [T, D], F32, kind="ExternalInput")
    wq = nc.dram_tensor("wq", [D, 256], F32R, kind="ExternalInput")
    wk = nc.dram_tensor("wk", [D, 256], F32R, kind="ExternalInput")
    wv = nc.dram_tensor("wv", [D, 256], F32R, kind="ExternalInput")
    wo = nc.dram_tensor("wo", [256, D], F32R, kind="ExternalInput")
    wi = nc.dram_tensor("wi", [D, 324], F32, kind="ExternalInput")
    out = nc.dram_tensor("out", [T, D], F32, kind="ExternalOutput")
    with tile.TileContext(nc) as tc:
        if loop:
            with tc.For_i(0, loop, 1):
                build_kernel(tc, out.ap(), x.ap(), wq.ap(), wk.ap(), wv.ap(), wo.ap(), wi.ap())
        else:
            build_kernel(tc, out.ap(), x.ap(), wq.ap(), wk.ap(), wv.ap(), wo.ap(), wi.ap())
    nc.compile()
    return nc


def kernel(x, Wq, Wk, Wv, Wo, bo, Wqi, Wki, Ww, _trace=False):
    _install_neff_cache()
    x, Wq, Wk, Wv, Wo, bo, Wqi, Wki, Ww = (
        np.ascontiguousarray(np.asarray(a, np.float32))
        for a in (x, Wq, Wk, Wv, Wo, bo, Wqi, Wki, Ww))
    nc = _build_nc()
    in_maps = _make_in_maps(x, Wq, Wk, Wv, Wo, Wqi, Wki, Ww)
    res = run_bass_kernel_spmd(nc, in_maps, core_ids=list(range(8)), trace=_trace)
    outs = [r["out"] for r in res.results]
    full = np.stack([sum(outs[b * 4:(b + 1) * 4]) + bo for b in range(B)], axis=0)
    full = full.astype(np.float32)
    if _trace:
        return full, res
    return full


def _make_in_maps(x, Wq, Wk, Wv, Wo, Wqi, Wki, Ww):
    wi = np.ascontiguousarray(np.concatenate([Wqi, Wki, Ww], axis=1))
    in_maps = []
    for b in range(B):
        for g in range(4):
            c = slice(g * 256, (g + 1) * 256)
            in_maps.append({
                "x": np.ascontiguousarray(x[b]),
                "wq": np.ascontiguousarray(Wq[:, c]),
                "wk": np.ascontiguousarray(Wk[:, c]),
                "wv": np.ascontiguousarray(Wv[:, c]),
                "wo": np.ascontiguousarray(Wo[c, :]),
                "wi": wi,
            })
    return in_maps


def bench_exec_ns(inputs, iters=10, loop=256):
    """Per-iteration device time: the kernel body loops `loop` times inside one
    NEFF; dispatch-overhead floor is subtracted via the slope between two loop
    counts. Returns ns per kernel iteration."""
    lo = max(1, loop // 8)
    t_hi = _bench_exec_wall(inputs, iters, loop)
    t_lo = _bench_exec_wall(inputs, iters, lo)
    return (t_hi - t_lo) / (loop - lo) * 1e9


def _bench_exec_wall(inputs, iters, loop):
    import time

    import jax
    from jax.experimental.shard_map import shard_map
    from jax.sharding import Mesh, NamedSharding, PartitionSpec

    import concourse.bass2jax as b2j

    _install_neff_cache()
    b2j.install_neuronx_cc_hook()
    nc = _build_nc(loop=loop)
    ins = {k: np.ascontiguousarray(np.asarray(v, np.float32)) for k, v in inputs.items()}
    in_maps = _make_in_maps(ins["x"], ins["Wq"], ins["Wk"], ins["Wv"], ins["Wo"],
                            ins["Wqi"], ins["Wki"], ins["Ww"])

    partition_name = nc.partition_id_tensor.name if nc.partition_id_tensor else None
    in_names, out_names, out_avals, zero_outs = [], [], [], []
    for alloc in nc.m.functions[0].allocations:
        if not isinstance(alloc, mybir.MemoryLocationSet):
            continue
        name = alloc.memorylocations[0].name
        if alloc.kind == "ExternalInput":
            if name != partition_name:
                in_names.append(name)
        elif alloc.kind == "ExternalOutput":
            shape = tuple(alloc.tensor_shape)
            dtype = mybir.dt.np(alloc.dtype)
            out_names.append(name)
            out_avals.append(jax.core.ShapedArray(shape, dtype))
            zero_outs.append(np.zeros(shape, dtype))
    n_params = len(in_names)
    all_in_names = list(in_names) + list(out_names)
    if partition_name is not None:
        all_in_names.append(partition_name)

    def _body(*args):
        operands = list(args)
        if partition_name is not None:
            operands.append(b2j.partition_id_tensor())
        outs = b2j._bass_exec_p.bind(
            *operands,
            out_avals=tuple(out_avals),
            in_names=tuple(all_in_names),
            out_names=tuple(out_names),
            lowering_input_output_aliases=(),
            sim_require_finite=True,
            sim_require_nnan=True,
            nc=nc,
        )
        return tuple(outs)

    n_cores = len(in_maps)
    devices = jax.devices()[:n_cores]
    mesh = Mesh(np.asarray(devices), ("core",))
    in_specs = (PartitionSpec("core"),) * (n_params + len(out_names))
    out_specs = (PartitionSpec("core"),) * len(out_names)
    fn = jax.jit(shard_map(_body, mesh=mesh, in_specs=in_specs,
                           out_specs=out_specs, check_rep=False))
    sharding = NamedSharding(mesh, PartitionSpec("core"))
    dev_args = [
        jax.device_put(
            np.concatenate([np.asarray(in_maps[c][nm]) for c in range(n_cores)], axis=0),
            sharding)
        for nm in in_names
    ] + [
        jax.device_put(np.concatenate([z] * n_cores, axis=0), sharding)
        for z in zero_outs
    ]
    r = fn(*dev_args)
    jax.block_until_ready(r)
    times = []
    for _ in range(iters):
        t0 = time.perf_counter()
        r = fn(*dev_args)
        jax.block_until_ready(r)
        times.append(time.perf_counter() - t0)
    return min(times)


if __name__ == "__main__":
    rng = np.random.default_rng(0)
    ins = {
        "x": rng.standard_normal((B, T, D)).astype(np.float32),
        "Wq": (rng.standard_normal((D, D)) * 0.02).astype(np.float32),
        "Wk": (rng.standard_normal((D, D)) * 0.02).astype(np.float32),
        "Wv": (rng.standard_normal((D, D)) * 0.02).astype(np.float32),
        "Wo": (rng.standard_normal((D, D)) * 0.02).astype(np.float32),
        "bo": np.zeros(D, np.float32),
        "Wqi": (rng.standard_normal((D, HI * IHD)) * 0.02).astype(np.float32),
        "Wki": (rng.standard_normal((D, IHD)) * 0.02).astype(np.float32),
        "Ww": (rng.standard_normal((D, HI)) * 0.02).astype(np.float32),
    }
    out = kernel(**ins)
    print("out", out.shape, out.dtype, float(np.abs(out).max()))
